# revision 25
# baseline (speedup 1.0000x reference)
"""Trainium2 Bass kernel for BasicLSTM (nn_BasicLSTM_16320875724833).

Problem: inputs [256, 1024, 128] f32; LSTM(H=256) over T=1024 steps, then
linear [256->2] + softmax on the final hidden state. Output [256, 2] f32.

v2b — latency-oriented redesign. The LSTM step is a serial dependency
chain (batch elements all traverse the same 1024 sequential steps), so
the kernel minimizes per-step chain latency rather than engine busy:

  - 8 cores data-parallel over batch (32 rows/core), state feature-major.
  - All four gate nonlinearities in ONE Activation on the critical path:
    sigmoid for i,f and tanh(g) = 2*sigmoid(2g)-1 with the 2x pre-scale
    folded into W_ih/W_hh/bias rows of g (host-side). The o gates go to
    a SEPARATE PSUM tile + separate sigmoid that runs off the critical
    path (its matmuls are emitted last).
  - Cell state kept HALVED (ch = c/2), making the update a pure fused
    chain with no scalar fixups:
      P1 = (S_g - 0.5) * S_i        # scalar_tensor_tensor (DVE)
      P2 = S_f * ch                  # tensor mult (DVE)
      ch' = P1 + P2                  # tensor add (DVE)
      tanh(c') = tanh(2*ch')         # ACT free pre-scale
      h = S_o * tanh(c')             # tensor mult (DVE)
  - Recurrence matmuls kc-major, g/i/f chunks first, o chunks last, so
    the critical-path sigmoid starts after 12 of 16 matmuls.
  - fp8(e4m3) W_hh stationary (halves LDWEIGHTS); fp16 h state.
  - Input projection hoisted: x DMA-transposed to d-major once, then
    per-4-step-group matmuls accumulate Wx + bias into PSUM banks
    (K=4/K=2 indicator-matmul bias fill).
  - Head: softmax over 2 classes == [sigmoid(d), sigmoid(-d)] with
    d = h @ (W_lin[0]-W_lin[1]) + (b_lin[0]-b_lin[1]).
"""

import numpy as np

# ---- problem constants (hardcoded; kernel.py must be self-contained) ----
B, T, D, H = 256, 1024, 128, 256
NCORES = 8
BLOC = B // NCORES          # 32 batch rows per core
GC = 8                      # gate chunks of 128 (4H = 1024)
KC = 2                      # hidden chunks of 128 (H = 256)
G4 = 4                      # timesteps per PSUM group

W8 = True                   # fp8(e4m3) W_hh stationary operand
W8IH = False                # fp8(e4m3) W_ih projection stationary operand
GBUFS = 2                   # PSUM pool buffers per gates tile
P2ENG = "dve"               # engine for P2 = S_f * ch: "pool" or "dve"
HSPLIT = False              # split tanh/h-mul per kc half
HONE = True                 # single hT tile + one h-mul (vs hT0/hT1 + two)
TTMAJ = False               # tt-major PSUM layout (contiguous ACT reads)
SIG2 = False                # split sigmoid: g,i after 8 MMs; f after 12
REPEAT = 1                  # timing-only: run the recurrence REPEAT times

_cache = {}


def _build_program(seq_len=T, debug_state=False):
    import concourse.bass as bass
    import concourse.mybir as mybir
    from concourse import bacc
    from concourse.tile import TileContext
    from contextlib import ExitStack

    f16 = mybir.dt.float16
    f32 = mybir.dt.float32
    AF = mybir.ActivationFunctionType
    ALU = mybir.AluOpType

    nc = bacc.Bacc(None, target_bir_lowering=False)

    x = nc.dram_tensor("x", [BLOC, seq_len, D], f16, kind="ExternalInput")
    ihdt = mybir.dt.float8e4 if W8IH else f16
    wih = nc.dram_tensor("wih", [D, 4 * H], ihdt, kind="ExternalInput")
    wdt = mybir.dt.float8e4 if W8 else f16
    whh = nc.dram_tensor("whh", [128, KC, 4 * H], wdt, kind="ExternalInput")
    biasA = nc.dram_tensor("biasA", [4, 128], f16, kind="ExternalInput")
    biasB = nc.dram_tensor("biasB", [2, 128], f16, kind="ExternalInput")
    biasO = nc.dram_tensor("biasO", [2, 128], f16, kind="ExternalInput")
    ind4 = nc.dram_tensor("ind4", [4, 4 * G4 * BLOC], f16, kind="ExternalInput")
    ind2 = nc.dram_tensor("ind2", [2, 2 * G4 * BLOC], f16, kind="ExternalInput")
    wd = nc.dram_tensor("wd", [128, KC, 1], f16, kind="ExternalInput")
    out = nc.dram_tensor("out", [1, 2, BLOC], f32, kind="ExternalOutput")
    if debug_state:
        dbg_h = nc.dram_tensor("dbg_h", [128, KC, BLOC], f32, kind="ExternalOutput")
        dbg_ch = nc.dram_tensor("dbg_ch", [128, KC, BLOC], f32, kind="ExternalOutput")
        dbg_S = nc.dram_tensor("dbg_S", [128, 6, BLOC], f32, kind="ExternalOutput")
        dbg_So = nc.dram_tensor("dbg_So", [128, 2, BLOC], f32, kind="ExternalOutput")
        dbg_G = nc.dram_tensor("dbg_G", [128, 6, BLOC], f32, kind="ExternalOutput")

    with ExitStack() as ctx:
        tc = ctx.enter_context(TileContext(nc))
        consts = ctx.enter_context(tc.tile_pool(name="consts", bufs=1))
        state = ctx.enter_context(tc.tile_pool(name="state", bufs=1))
        xbp = ctx.enter_context(tc.tile_pool(name="xbp", bufs=1))
        ew = ctx.enter_context(tc.tile_pool(name="ew", bufs=3))
        gpsum = ctx.enter_context(tc.tile_pool(name="gpsum", bufs=GBUFS, space="PSUM"))
        opsum = ctx.enter_context(tc.tile_pool(name="opsum", bufs=GBUFS, space="PSUM"))
        hpsum = ctx.enter_context(tc.tile_pool(name="hpsum", bufs=1, space="PSUM"))

        # constants into SBUF
        wih_sb = consts.tile([128, 4 * H], ihdt)
        nc.sync.dma_start(out=wih_sb[:, :], in_=wih[:, :])
        whh_sb = consts.tile([128, KC, 4 * H], wdt)
        nc.sync.dma_start(out=whh_sb[:, :, :], in_=whh[:, :, :])
        biasA_sb = consts.tile([4, 128], f16)
        nc.sync.dma_start(out=biasA_sb[:, :], in_=biasA[:, :])
        biasB_sb = consts.tile([2, 128], f16)
        nc.sync.dma_start(out=biasB_sb[:, :], in_=biasB[:, :])
        biasO_sb = consts.tile([2, 128], f16)
        nc.sync.dma_start(out=biasO_sb[:, :], in_=biasO[:, :])
        ind4_sb = consts.tile([4, 4 * G4 * BLOC], f16)
        nc.sync.dma_start(out=ind4_sb[:, :], in_=ind4[:, :])
        ind2_sb = consts.tile([2, 2 * G4 * BLOC], f16)
        nc.sync.dma_start(out=ind2_sb[:, :], in_=ind2[:, :])
        wd_sb = consts.tile([128, KC, 1], f16)
        nc.sync.dma_start(out=wd_sb[:, :, :], in_=wd[:, :, :])

        # persistent state (feature-major); ch = c/2
        if HONE:
            hT = state.tile([128, KC, BLOC], f16)
            nc.vector.memset(hT[:, :, :], 0.0)
            hrhs = [hT[:, 0, :], hT[:, 1, :]]
            hhead = hrhs
        else:
            hT0 = state.tile([128, BLOC], f16)
            hT1 = state.tile([128, BLOC], f16)
            nc.vector.memset(hT0[:, :], 0.0)
            nc.vector.memset(hT1[:, :], 0.0)
            hrhs = [hT0[:, :], hT1[:, :]]
            hhead = hrhs
        ch = state.tile([128, KC, BLOC], f32)
        nc.vector.memset(ch[:, :, :], 0.0)

        # one giant xbar-transpose of the whole input: [BLOC*T, D] -> [D, BLOC*T]
        xTb = xbp.tile([128, BLOC, seq_len], f16)
        nc.sync.dma_start_transpose(
            out=xTb[:, :, :],
            in_=x[:, :, :].rearrange("b t d -> (b t) d"),
        )

        for gi in range(REPEAT * seq_len // G4):
            t0 = (gi * G4) % seq_len
            # gif tile: chunks 0..5 = [g0,g1,i0,i1,f0,f1]; o tile: [o0,o1]
            if TTMAJ:
                Pr = gpsum.tile([128, G4, 6, BLOC], f32, tag="gif")
                Por = opsum.tile([128, G4, 2, BLOC], f32, tag="o")
                Pv = Pr.rearrange("p t c b -> p c t b")
                Pov = Por.rearrange("p t c b -> p c t b")
                bA, bB, bO = Pr[:, :, 0:4, :], Pr[:, :, 4:6, :], Por[:, :, :, :]
            else:
                Pr = gpsum.tile([128, 6, G4, BLOC], f32, tag="gif")
                Por = opsum.tile([128, 2, G4, BLOC], f32, tag="o")
                Pv, Pov = Pr, Por
                bA, bB, bO = Pr[:, 0:4, :, :], Pr[:, 4:6, :, :], Por[:, :, :, :]
            # bias fill via indicator matmuls (start=True)
            nc.tensor.matmul(bA, lhsT=biasA_sb[:, :], rhs=ind4_sb[:, :],
                             start=True, stop=False, skip_group_check=True)
            nc.tensor.matmul(bB, lhsT=biasB_sb[:, :], rhs=ind2_sb[:, :],
                             start=True, stop=False, skip_group_check=True)
            nc.tensor.matmul(bO, lhsT=biasO_sb[:, :], rhs=ind2_sb[:, :],
                             start=True, stop=False, skip_group_check=True)
            # input projection for these 4 steps (strided t-major view of xTb)
            xvw = xTb[:, :, t0:t0 + G4].rearrange("p b t -> p t b")
            for gc in range(6):
                nc.tensor.matmul(Pv[:, gc, :, :],
                                 lhsT=wih_sb[:, gc * 128:(gc + 1) * 128],
                                 rhs=xvw, start=False, stop=False,
                                 skip_group_check=True)
            for oc in range(2):
                gc = 6 + oc
                nc.tensor.matmul(Pov[:, oc, :, :],
                                 lhsT=wih_sb[:, gc * 128:(gc + 1) * 128],
                                 rhs=xvw, start=False, stop=False,
                                 skip_group_check=True)
            for tt in range(G4):
                # recurrence matmuls: kc-major, g/i/f first, o last
                gc_blocks = [(0, 4), (4, 6)] if SIG2 else [(0, 6)]
                for lo, hi in gc_blocks:
                    for kc in range(KC):
                        for gc in range(lo, hi):
                            nc.tensor.matmul(
                                Pv[:, gc, tt, :],
                                lhsT=whh_sb[:, kc, gc * 128:(gc + 1) * 128],
                                rhs=hrhs[kc],
                                start=False, stop=(kc == KC - 1),
                                skip_group_check=True)
                for kc in range(KC):
                    for oc in range(2):
                        gc = 6 + oc
                        nc.tensor.matmul(
                            Pov[:, oc, tt, :],
                            lhsT=whh_sb[:, kc, gc * 128:(gc + 1) * 128],
                            rhs=hrhs[kc],
                            start=False, stop=(kc == KC - 1),
                            skip_group_check=True)
                # S = sigmoid over g,i,f chunks; So = sigmoid(o)
                S = ew.tile([128, 6, BLOC], f32, tag="S")
                sin_ = Pr[:, tt, :, :] if TTMAJ else Pr[:, :, tt, :]
                if SIG2:
                    nc.scalar.activation(S[:, 0:4, :], Pr[:, 0:4, tt, :],
                                         AF.Sigmoid)
                    nc.scalar.activation(S[:, 4:6, :], Pr[:, 4:6, tt, :],
                                         AF.Sigmoid)
                else:
                    nc.scalar.activation(S[:, :, :], sin_, AF.Sigmoid)
                if debug_state and gi == 0 and tt == 0:
                    _dbgS = consts.tile([128, 6, BLOC], f32)
                    _dbgSo = consts.tile([128, 2, BLOC], f32)
                    _dbgG = consts.tile([128, 6, BLOC], f32)
                    nc.vector.tensor_copy(_dbgS[:, :, :], S[:, :, :])
                    nc.vector.tensor_copy(_dbgG[:, :, :], sin_)
                So = ew.tile([128, 2, BLOC], f32, tag="So")
                soin = Por[:, tt, :, :] if TTMAJ else Por[:, :, tt, :]
                nc.scalar.activation(So[:, :, :], soin, AF.Sigmoid)
                if debug_state and gi == 0 and tt == 0:
                    nc.vector.tensor_copy(_dbgSo[:, :, :], So[:, :, :])
                # ch' = S_f*ch + S_i*(S_g - 0.5)   (ch = c/2)
                P1 = ew.tile([128, 2, BLOC], f32, tag="P1")
                nc.vector.scalar_tensor_tensor(
                    P1[:, :, :], S[:, 0:2, :], 0.5, S[:, 2:4, :],
                    ALU.subtract, ALU.mult)
                P2 = ew.tile([128, 2, BLOC], f32, tag="P2")
                p2eng = nc.gpsimd if P2ENG == "pool" else nc.vector
                p2eng.tensor_mul(P2[:, :, :], S[:, 4:6, :], ch[:, :, :])
                nc.vector.tensor_add(ch[:, :, :], P1[:, :, :], P2[:, :, :])
                # h = So * tanh(c) = So * tanh(2*ch)
                thc = ew.tile([128, 2, BLOC], f32, tag="thc")
                if HONE:
                    nc.scalar.activation(thc[:, :, :], ch[:, :, :], AF.Tanh,
                                         scale=2.0)
                    nc.vector.tensor_mul(hT[:, :, :], So[:, :, :], thc[:, :, :])
                elif HSPLIT:
                    for hc in range(KC):
                        nc.scalar.activation(thc[:, hc:hc + 1, :],
                                             ch[:, hc:hc + 1, :], AF.Tanh,
                                             scale=2.0)
                        nc.vector.tensor_mul(hrhs[hc],
                                             So[:, hc, :], thc[:, hc, :])
                else:
                    nc.scalar.activation(thc[:, :, :], ch[:, :, :], AF.Tanh,
                                         scale=2.0)
                    nc.vector.tensor_mul(hrhs[0], So[:, 0, :], thc[:, 0, :])
                    nc.vector.tensor_mul(hrhs[1], So[:, 1, :], thc[:, 1, :])

        # head: d = h @ w_d + b_d ; probs = [sigmoid(d+bd), sigmoid(-d-bd)]
        hps = hpsum.tile([1, BLOC], f32)
        nc.tensor.matmul(hps[:, :], lhsT=wd_sb[:, 0, :], rhs=hhead[0],
                         start=True, stop=False, skip_group_check=True)
        nc.tensor.matmul(hps[:, :], lhsT=wd_sb[:, 1, :], rhs=hhead[1],
                         start=False, stop=True, skip_group_check=True)
        outsb = consts.tile([1, 2, BLOC], f32)
        bd_pos = consts.tile([1, 1], f32)
        bd_neg = consts.tile([1, 1], f32)
        nc.vector.memset(bd_pos[:, :], float(_cache["b_d"]))
        nc.vector.memset(bd_neg[:, :], -float(_cache["b_d"]))
        nc.scalar.activation(outsb[:, 0, :], hps[:, :], AF.Sigmoid,
                             bias=bd_pos[:, :], scale=1.0)
        nc.scalar.activation(outsb[:, 1, :], hps[:, :], AF.Sigmoid,
                             bias=bd_neg[:, :], scale=-1.0)
        nc.sync.dma_start(out=out[:, :, :], in_=outsb[:, :, :])
        if debug_state:
            hdbg = consts.tile([128, KC, BLOC], f32)
            nc.vector.tensor_copy(hdbg[:, 0, :], hrhs[0])
            nc.vector.tensor_copy(hdbg[:, 1, :], hrhs[1])
            nc.sync.dma_start(out=dbg_h[:, :, :], in_=hdbg[:, :, :])
            nc.sync.dma_start(out=dbg_ch[:, :, :], in_=ch[:, :, :])
            nc.sync.dma_start(out=dbg_S[:, :, :], in_=_dbgS[:, :, :])
            nc.sync.dma_start(out=dbg_So[:, :, :], in_=_dbgSo[:, :, :])
            nc.sync.dma_start(out=dbg_G[:, :, :], in_=_dbgG[:, :, :])

    nc.compile()
    return nc


def _prep_host(inputs, W_ih, W_hh, b_ih, b_hh, W_lin, b_lin):
    """Host-side weight preprocessing: gate permutation, 2x fold for the
    g rows (tanh-as-sigmoid), transposed layouts."""
    import concourse.mybir as _mb
    # PyTorch gate row order [i, f, g, o] (256 each) -> chunk order
    # [g0, g1, i0, i1, f0, f1, o0, o1] (128-row chunks)
    perm = np.concatenate([
        np.arange(512, 768),    # g
        np.arange(0, 256),      # i
        np.arange(256, 512),    # f
        np.arange(768, 1024),   # o
    ])
    wnp = _mb.dt.np(_mb.dt.float8e4) if W8 else np.float16
    ihnp = _mb.dt.np(_mb.dt.float8e4) if W8IH else np.float16

    rs = np.ones((4 * H, 1), np.float32)
    rs[0:256] = 2.0             # g rows pre-scaled so tanh(g) = 2*sig(2g)-1

    Wih_p = np.ascontiguousarray(W_ih[perm] * rs)                   # [1024, 128]
    Whh_p = np.ascontiguousarray(W_hh[perm] * rs)                   # [1024, 256]
    b_p = ((b_ih + b_hh)[perm] * rs[:, 0]).astype(np.float32)       # [1024]

    wih_host = np.ascontiguousarray(Wih_p.T).astype(ihnp)           # [128, 1024]
    whh_host = np.ascontiguousarray(
        Whh_p.T.reshape(KC, 128, 4 * H).transpose(1, 0, 2)
    ).astype(wnp)                                                   # [128, 2, 1024]
    # bias lhsT for the indicator fills (chunks 0..3 / 4..5 / 6..7)
    bb = b_p.reshape(8, 128).astype(np.float16)
    biasA = np.ascontiguousarray(bb[0:4])
    biasB = np.ascontiguousarray(bb[4:6])
    biasO = np.ascontiguousarray(bb[6:8])
    seg = G4 * BLOC
    if TTMAJ:
        # out free order is [tt, chunk, b]: indicator selects the chunk
        ind4 = np.kron(np.ones((1, G4), np.float16),
                       np.kron(np.eye(4, dtype=np.float16),
                               np.ones((1, BLOC), np.float16)))
        ind2 = np.kron(np.ones((1, G4), np.float16),
                       np.kron(np.eye(2, dtype=np.float16),
                               np.ones((1, BLOC), np.float16)))
    else:
        ind4 = np.kron(np.eye(4, dtype=np.float16), np.ones((1, seg), np.float16))
        ind2 = np.kron(np.eye(2, dtype=np.float16), np.ones((1, seg), np.float16))
    w_d = (W_lin[0] - W_lin[1]).astype(np.float32)                  # [256]
    wd_host = np.ascontiguousarray(
        w_d.reshape(KC, 128).T.reshape(128, KC, 1)).astype(np.float16)
    b_d = float(b_lin[0] - b_lin[1])

    x_f16 = inputs.astype(np.float16)                               # [256, T, 128]
    host = {"wih": wih_host, "whh": whh_host, "biasA": biasA,
            "biasB": biasB, "biasO": biasO,
            "ind4": np.ascontiguousarray(ind4),
            "ind2": np.ascontiguousarray(ind2), "wd": wd_host}
    return x_f16, host, b_d


def kernel(inputs, W_ih, W_hh, b_ih, b_hh, W_lin, b_lin):
    from concourse.bass_utils import run_bass_kernel_spmd

    inputs = np.asarray(inputs, dtype=np.float32)
    x_f16, host, b_d = _prep_host(
        np.asarray(inputs), np.asarray(W_ih), np.asarray(W_hh),
        np.asarray(b_ih), np.asarray(b_hh), np.asarray(W_lin), np.asarray(b_lin))
    if _cache.get("b_d") != b_d or "nc" not in _cache:
        _cache["b_d"] = b_d
        _cache["nc"] = _build_program(T)
    nc = _cache["nc"]

    in_maps = []
    for j in range(NCORES):
        in_maps.append({
            "x": np.ascontiguousarray(x_f16[j * BLOC:(j + 1) * BLOC]), **host})

    res = run_bass_kernel_spmd(nc, in_maps, core_ids=list(range(NCORES)))
    _cache["last_result"] = res
    out = np.concatenate(
        [np.asarray(r["out"])[0].T for r in res.results], axis=0)
    return np.ascontiguousarray(out).astype(np.float32)
